# revision 1
# baseline (speedup 1.0000x reference)
"""Trainium2 Bass kernel for nn_MultiHeadSelfAttention_36472862277880, v2.

Sparse attention (local window 128, global stride 64, causal) with RoPE.
Sharding: 8 cores = 4 batches x 2 head-groups (8 heads each core).

v2 redesign vs baseline:
- x transposed on host -> no PE transposes / staging copies; DMA straight
  into bf16 lhsT tiles.
- all matmuls bf16 (FWL weight loads), psum f32.
- QK projection emits 4-head-packed E/O tiles [128 = 4 heads x 32 freqs]:
  rope runs full-128-partition DVE ops (half the op count), and scores use
  two K=32 row-strip matmuls per chunk (tile_position) -> no gpsimd
  deinterleave copies at all.
- global-key scores packed 4 heads per psum bank (col strips).
- softmax reciprocal via reciprocal_approx_fast (single DVE op).

Self-contained: hardcodes all shapes; only imports the system concourse repo.
"""
import sys

if "/opt/trn_rl_repo" not in sys.path:
    sys.path.insert(0, "/opt/trn_rl_repo")

import numpy as np
import ml_dtypes

import concourse.bass as bass
import concourse.bacc as bacc
import concourse.tile as tile
from concourse import mybir
from concourse.bass_utils import run_bass_kernel_spmd
from concourse.masks import make_identity

F32 = mybir.dt.float32
F32R = mybir.dt.float32r
BF16 = mybir.dt.bfloat16

B, T, D, H, DH = 4, 2048, 1024, 16, 64
HALF = T // 2
NCORES = 8
EXP_SCALE = 0.125
EXP = None  # set in _build

_cache = {}


def _build():
    nc = bacc.Bacc("TRN2", target_bir_lowering=False, debug=False, num_devices=1)

    xt_d = nc.dram_tensor("xt", [128, 8, T], BF16, kind="ExternalInput").ap()
    xg_d = nc.dram_tensor("xg", [128, 8, 32], BF16, kind="ExternalInput").ap()
    wq_d = nc.dram_tensor("wq", [128, 2, 2, 8, 128], BF16, kind="ExternalInput").ap()
    wk_d = nc.dram_tensor("wk", [128, 2, 2, 8, 128], BF16, kind="ExternalInput").ap()
    wv_d = nc.dram_tensor("wv", [128, 8, 512], BF16, kind="ExternalInput").ap()
    wo_d = nc.dram_tensor("wo", [128, 4, 1024], BF16, kind="ExternalInput").ap()
    cos_d = nc.dram_tensor("cos4", [128, T], F32, kind="ExternalInput").ap()
    sin_d = nc.dram_tensor("sin4", [128, T], F32, kind="ExternalInput").ap()
    cosg_d = nc.dram_tensor("cosg", [128, 32], F32, kind="ExternalInput").ap()
    sing_d = nc.dram_tensor("sing", [128, 32], F32, kind="ExternalInput").ap()
    mwin_d = nc.dram_tensor("mwin", [128, 512], BF16, kind="ExternalInput").ap()
    mwin4_d = nc.dram_tensor("mwin4", [128, 2048], BF16, kind="ExternalInput").ap()
    mglob_d = nc.dram_tensor("mglob4", [128, T], BF16, kind="ExternalInput").ap()
    mp2_d = nc.dram_tensor("mp2", [128, 512], BF16, kind="ExternalInput").ap()
    out_d = nc.dram_tensor("out", [T, D], F32, kind="ExternalOutput").ap()

    EXPF = mybir.ActivationFunctionType.Exp

    with tile.TileContext(nc) as tc:
        from contextlib import ExitStack
        with ExitStack() as ctx:
            sb = ctx.enter_context(tc.tile_pool(name="sb", bufs=1))
            psqk = ctx.enter_context(tc.tile_pool(name="psqk", bufs=1, space="PSUM"))
            pss = ctx.enter_context(tc.tile_pool(name="pss", bufs=1, space="PSUM"))
            psu = ctx.enter_context(tc.tile_pool(name="psu", bufs=1, space="PSUM"))

            # ---------- constants / weights ----------
            ident = sb.tile([32, 32], F32, name="ident")
            make_identity(nc, ident[:])
            onesr = sb.tile([1, 64], F32, name="onesf")
            nc.vector.memset(onesr[:], 1.0)

            wv = sb.tile([128, 8, 512], BF16, name="wv")
            nc.sync.dma_start(out=wv[:], in_=wv_d)
            xg = sb.tile([128, 8, 32], BF16, name="xg")
            nc.sync.dma_start(out=xg[:], in_=xg_d)
            cosg = sb.tile([128, 32], F32, name="cosg")
            nc.sync.dma_start(out=cosg[:], in_=cosg_d)
            sing = sb.tile([128, 32], F32, name="sing")
            nc.sync.dma_start(out=sing[:], in_=sing_d)
            wq = sb.tile([128, 2, 2, 8, 128], BF16, name="wq")
            nc.sync.dma_start(out=wq[:], in_=wq_d)
            wk = sb.tile([128, 2, 2, 8, 128], BF16, name="wk")
            nc.sync.dma_start(out=wk[:], in_=wk_d)
            cos4 = sb.tile([128, T], F32, name="cos4")
            nc.sync.dma_start(out=cos4[:], in_=cos_d)
            sin4 = sb.tile([128, T], F32, name="sin4")
            nc.sync.dma_start(out=sin4[:], in_=sin_d)

            # ---------- early pass: globals (t = 64m) ----------

            # V at global tokens
            pvg = pss.tile([128, 512], F32, tag="pss", bufs=3, name="pvg")
            for dc in range(8):
                nc.tensor.matmul(pvg[0:32, :], xg[:, dc, :], wv[:, dc, :],
                                 start=(dc == 0), stop=(dc == 7))
            vg = sb.tile([32, 8, 65], BF16, name="vg")
            nc.scalar.copy(vg[:, :, 0:64],
                           pvg[0:32, :].rearrange("p (a b) -> p a b", a=8))
            nc.vector.memset(vg[:, :, 64], 1.0)

            # Q,K at global tokens: packed [128 = 4h x 32f, 32 gtok] per (qk, G, eo)
            pall = pss.tile([128, 16, 32], F32, tag="pss", bufs=3, name="pall")
            for qk, wd in ((0, wq), (1, wk)):
                for G in range(2):
                    for eo in range(2):
                        idx = 4 * qk + 2 * G + eo
                        for dc in range(8):
                            nc.tensor.matmul(
                                pall[:, idx, :], wd[:, G, eo, dc, :], xg[:, dc, :],
                                start=(idx == 0 and dc == 0),
                                stop=(idx == 7 and dc == 7),
                                skip_group_check=True)

            qgE, qgO, kgE, kgO = {}, {}, {}, {}
            rtg = [sb.tile([128, 32], F32, tag="rtg", bufs=4, name=f"rtg{i}")
                   for i in range(4)]
            for qk, (Ed, Od) in ((0, (qgE, qgO)), (1, (kgE, kgO))):
                for G in range(2):
                    pE = pall[:, 4 * qk + 2 * G + 0, :]
                    pO = pall[:, 4 * qk + 2 * G + 1, :]
                    Et = sb.tile([128, 32], BF16, tag="qgE", bufs=8,
                                 name=f"g{qk}{G}E")
                    Ot = sb.tile([128, 32], BF16, tag="qgO", bufs=8,
                                 name=f"g{qk}{G}O")
                    t0_, t1_ = rtg[0], rtg[1]
                    nc.vector.tensor_mul(t0_[:], pE, cosg[:])
                    nc.vector.tensor_mul(t1_[:], pO, sing[:])
                    nc.vector.tensor_sub(Et[:], t0_[:], t1_[:])
                    t2_, t3_ = rtg[2], rtg[3]
                    nc.vector.tensor_mul(t2_[:], pE, sing[:])
                    nc.vector.tensor_mul(t3_[:], pO, cosg[:])
                    nc.vector.tensor_add(Ot[:], t2_[:], t3_[:])
                    Ed[G], Od[G] = Et, Ot

            mwin = sb.tile([128, 512], BF16, name="mwin")
            nc.sync.dma_start(out=mwin[:], in_=mwin_d)
            mwin4 = sb.tile([128, 2048], BF16, name="mwin4")
            nc.sync.dma_start(out=mwin4[:], in_=mwin4_d)
            mglob4 = sb.tile([128, T], BF16, name="mglob4")
            nc.sync.dma_start(out=mglob4[:], in_=mglob_d)
            mp2 = sb.tile([128, 512], BF16, name="mp2")
            nc.sync.dma_start(out=mp2[:], in_=mp2_d)
            wo = sb.tile([128, 4, 1024], BF16, name="wo")
            nc.sync.dma_start(out=wo[:], in_=wo_d)

            p2sb = [sb.tile([32, 65], F32, tag="p2sb", bufs=8, name=f"p2sb{h}")
                    for h in range(8)]
            ktbE, ktbO = {}, {}
            vaprev = None
            attn = None

            # ---------- half loop ----------
            for Hh in range(2):
                t0 = HALF * Hh

                xh = sb.tile([128, 8, HALF], BF16, tag="xh", bufs=2,
                             name=f"xh{Hh}")
                nc.sync.dma_start(out=xh[:], in_=xt_d[:, :, t0:t0 + HALF])

                # V projection
                va = []
                for jl in range(8):
                    pv = psu.tile([128, 512], F32, tag="psu", bufs=2,
                                  name=f"pv{Hh}_{jl}")
                    for dc in range(8):
                        nc.tensor.matmul(pv[:],
                                         xh[:, dc, 128 * jl:128 * jl + 128],
                                         wv[:, dc, :],
                                         start=(dc == 0), stop=(dc == 7))
                    v = sb.tile([128, 8, 65], BF16, tag="va", bufs=9,
                                name=f"va{Hh}_{jl}")
                    nc.scalar.copy(v[:, :, 0:64],
                                   pv[:].rearrange("p (a b) -> p a b", a=8))
                    nc.vector.memset(v[:, :, 64], 1.0)
                    va.append(v)

                # QK projection + rope -> qE/qO/kE/kO [128, 1024] bf16 per G
                qE, qO, kE, kO = {}, {}, {}, {}
                for G in range(2):
                    qE[G] = sb.tile([128, HALF], BF16, tag="qE", bufs=2,
                                    name=f"qE{Hh}_{G}")
                    qO[G] = sb.tile([128, HALF], BF16, tag="qO", bufs=2,
                                    name=f"qO{Hh}_{G}")
                    kE[G] = sb.tile([128, HALF], BF16, tag="kE", bufs=2,
                                    name=f"kE{Hh}_{G}")
                    kO[G] = sb.tile([128, HALF], BF16, tag="kO", bufs=2,
                                    name=f"kO{Hh}_{G}")
                    for tc_ in range(2):
                        sl = slice(512 * tc_, 512 * tc_ + 512)
                        cosS = cos4[:, t0 + 512 * tc_:t0 + 512 * tc_ + 512]
                        sinS = sin4[:, t0 + 512 * tc_:t0 + 512 * tc_ + 512]
                        for wd, Et, Ot in ((wq, qE, qO), (wk, kE, kO)):
                            pE = psqk.tile([128, 512], F32, tag="psqk", bufs=3,
                                           name=f"pE{Hh}_{G}_{tc_}_{id(wd)}")
                            pO = psqk.tile([128, 512], F32, tag="psqk", bufs=3,
                                           name=f"pO{Hh}_{G}_{tc_}_{id(wd)}")
                            for dc in range(8):
                                nc.tensor.matmul(pE[:], wd[:, G, 0, dc, :],
                                                 xh[:, dc, sl],
                                                 start=(dc == 0), stop=(dc == 7))
                            for dc in range(8):
                                nc.tensor.matmul(pO[:], wd[:, G, 1, dc, :],
                                                 xh[:, dc, sl],
                                                 start=(dc == 0), stop=(dc == 7))
                            ta = sb.tile([128, 512], F32, tag="rt", bufs=4,
                                         name=f"rta{Hh}_{G}_{tc_}")
                            tb = sb.tile([128, 512], F32, tag="rt", bufs=4,
                                         name=f"rtb{Hh}_{G}_{tc_}")
                            nc.vector.tensor_mul(ta[:], pE[:], cosS)
                            nc.vector.tensor_mul(tb[:], pO[:], sinS)
                            nc.vector.tensor_sub(Et[G][:, sl], ta[:], tb[:])
                            tc2 = sb.tile([128, 512], F32, tag="rt", bufs=4,
                                          name=f"rtc{Hh}_{G}_{tc_}")
                            td = sb.tile([128, 512], F32, tag="rt", bufs=4,
                                         name=f"rtd{Hh}_{G}_{tc_}")
                            nc.vector.tensor_mul(tc2[:], pE[:], sinS)
                            nc.vector.tensor_mul(td[:], pO[:], cosS)
                            nc.vector.tensor_add(Ot[G][:, sl], tc2[:], td[:])

                if Hh == 0:
                    for G in range(2):
                        ktbE[G] = sb.tile([128, 128], BF16, tag="ktbE", bufs=2,
                                          name=f"ktbE{G}")
                        nc.vector.tensor_copy(ktbE[G][:], kE[G][:, 896:1024])
                        ktbO[G] = sb.tile([128, 128], BF16, tag="ktbO", bufs=2,
                                          name=f"ktbO{G}")
                        nc.vector.tensor_copy(ktbO[G][:], kO[G][:, 896:1024])

                attn = [sb.tile([128, HALF], BF16, tag="attn", bufs=8,
                                name=f"attn{Hh}_{cc}") for cc in range(4)]

                for G in range(2):
                    for j in range(4):
                        h = 4 * G + j
                        cc, hh = h // 2, h % 2
                        rs = slice(32 * j, 32 * j + 32)
                        tp = (32 * j, 0)

                        # window scores (merged exp buffer, single mask mul)
                        et4 = sb.tile([128, 2048], BF16, tag="et4", bufs=2,
                                      name=f"et4{Hh}_{h}")
                        nc.vector.memset(et4[:, 1920:2048], 0.0)
                        for jp in range(4):
                            j0, j1 = 2 * jp, 2 * jp + 1
                            w1 = 256 if j1 < 7 else 128
                            pw = pss.tile([128, 512], F32, tag="pss", bufs=3,
                                          name=f"pw{Hh}_{h}_{jp}")
                            nc.tensor.matmul(
                                pw[:, 0:256],
                                kE[G][rs, 128 * j0:128 * j0 + 128],
                                qE[G][rs, 128 * j0:128 * j0 + 256],
                                start=True, stop=False, tile_position=tp,
                                skip_group_check=True)
                            nc.tensor.matmul(
                                pw[:, 0:256],
                                kO[G][rs, 128 * j0:128 * j0 + 128],
                                qO[G][rs, 128 * j0:128 * j0 + 256],
                                start=False, stop=False, tile_position=tp,
                                skip_group_check=True)
                            nc.tensor.matmul(
                                pw[:, 256:256 + w1],
                                kE[G][rs, 128 * j1:128 * j1 + 128],
                                qE[G][rs, 128 * j1:128 * j1 + w1],
                                start=False, stop=False, tile_position=tp,
                                skip_group_check=True)
                            nc.tensor.matmul(
                                pw[:, 256:256 + w1],
                                kO[G][rs, 128 * j1:128 * j1 + 128],
                                qO[G][rs, 128 * j1:128 * j1 + w1],
                                start=False, stop=True, tile_position=tp,
                                skip_group_check=True)
                            nc.scalar.activation(
                                et4[:, 512 * jp:512 * jp + 256 + w1],
                                pw[:, 0:256 + w1], EXPF, scale=EXP_SCALE)
                        ep4 = sb.tile([128, 2048], BF16, tag="ep4", bufs=2,
                                      name=f"ep4{Hh}_{h}")
                        nc.vector.tensor_mul(ep4[:], et4[:], mwin4[:])
                        ej = [ep4[:, 256 * i:256 * i + 256] for i in range(8)]

                        eb = None
                        if Hh == 1:
                            pb = pss.tile([128, 512], F32, tag="pss", bufs=3,
                                          name=f"pb{h}")
                            nc.tensor.matmul(pb[:, 0:128], ktbE[G][rs, :],
                                             qE[G][rs, 0:128],
                                             start=True, stop=False,
                                             tile_position=tp,
                                             skip_group_check=True)
                            nc.tensor.matmul(pb[:, 0:128], ktbO[G][rs, :],
                                             qO[G][rs, 0:128],
                                             start=False, stop=True,
                                             tile_position=tp,
                                             skip_group_check=True)
                            etb = sb.tile([128, 512], BF16, tag="et", bufs=3,
                                          name=f"etb{h}")
                            nc.scalar.activation(etb[:, 0:128], pb[:, 0:128],
                                                 EXPF, scale=EXP_SCALE)
                            eb = sb.tile([128, 128], BF16, tag="eb", bufs=2,
                                         name=f"eb{h}")
                            nc.vector.tensor_mul(eb[:], etb[:, 0:128],
                                                 mwin[:, 128:256])

                        # global-key scores
                        eglob = sb.tile([32, HALF], BF16, tag="eglob", bufs=2,
                                        name=f"eg{Hh}_{h}")
                        etg = sb.tile([32, HALF], BF16, tag="etg", bufs=2,
                                      name=f"etg{Hh}_{h}")
                        for tc_ in range(2):
                            sl = slice(512 * tc_, 512 * tc_ + 512)
                            pg = pss.tile([128, 512], F32, tag="pss", bufs=3,
                                          name=f"pg{Hh}_{h}_{tc_}")
                            nc.tensor.matmul(pg[0:32, :], kgE[G][rs, :],
                                             qE[G][rs, sl],
                                             start=True, stop=False,
                                             tile_position=tp,
                                             skip_group_check=True)
                            nc.tensor.matmul(pg[0:32, :], kgO[G][rs, :],
                                             qO[G][rs, sl],
                                             start=False, stop=True,
                                             tile_position=tp,
                                             skip_group_check=True)
                            nc.scalar.activation(etg[:, sl], pg[0:32, :],
                                                 EXPF, scale=EXP_SCALE)
                        nc.vector.tensor_mul(
                            eglob[:], etg[:],
                            mglob4[0:32, t0:t0 + HALF])

                        # global-query (p2) scores
                        pp = pss.tile([128, 512], F32, tag="pss", bufs=3,
                                      name=f"pp{Hh}_{h}")
                        for jl in range(8):
                            nc.tensor.matmul(
                                pp[:, 32 * jl:32 * jl + 32],
                                kE[G][rs, 128 * jl:128 * jl + 128],
                                qgE[G][rs, :],
                                start=(jl == 0), stop=False,
                                tile_position=tp, skip_group_check=True)
                            nc.tensor.matmul(
                                pp[:, 32 * jl:32 * jl + 32],
                                kO[G][rs, 128 * jl:128 * jl + 128],
                                qgO[G][rs, :],
                                start=False, stop=(jl == 7),
                                tile_position=tp, skip_group_check=True)
                        etp = sb.tile([128, 512], BF16, tag="et", bufs=3,
                                      name=f"etp{Hh}_{h}")
                        nc.scalar.activation(etp[:, 0:256], pp[:, 0:256],
                                             EXPF, scale=EXP_SCALE)
                        ep2 = sb.tile([128, 256], BF16, tag="ep2", bufs=2,
                                      name=f"ep2{Hh}_{h}")
                        nc.vector.tensor_mul(ep2[:], etp[:, 0:256],
                                             mp2[:, 256 * Hh:256 * Hh + 256])

                        pc = pss.tile([32, 65], F32, tag="pss", bufs=3,
                                      name=f"pc{Hh}_{h}")
                        for jl in range(8):
                            nc.tensor.matmul(pc[:], ep2[:, 32 * jl:32 * jl + 32],
                                             va[jl][:, h, :],
                                             start=(jl == 0), stop=(jl == 7))
                        if Hh == 0:
                            nc.vector.tensor_copy(p2sb[h][:], pc[:])
                        else:
                            nc.vector.tensor_add(p2sb[h][:], p2sb[h][:], pc[:])

                        # AV u-chunks
                        for c in range(2):
                            U = psu.tile([128, 512], F32, tag="psu", bufs=2,
                                         name=f"U{Hh}_{h}_{c}")
                            if c == 0:
                                parts = [(0, 0, 256, 0), (1, 128, 384, 0),
                                         (2, 256, 512, 0), (3, 384, 512, 0)]
                            else:
                                parts = [(4, 0, 256, 0), (3, 0, 128, 128),
                                         (5, 128, 384, 0), (6, 256, 512, 0),
                                         (7, 384, 512, 0)]
                            first = True
                            for (jl, u0, u1, e0c) in parts:
                                wdt = u1 - u0
                                nc.tensor.matmul(U[0:65, u0:u1],
                                                 va[jl][:, h, :],
                                                 ej[jl][:, e0c:e0c + wdt],
                                                 start=first, stop=False,
                                                 skip_group_check=True)
                                first = False
                            if Hh == 1 and c == 0:
                                nc.tensor.matmul(U[0:65, 0:128],
                                                 vaprev[:, h, :], eb[:],
                                                 start=False, stop=False,
                                                 skip_group_check=True)
                            nc.tensor.matmul(U[0:65, :], vg[:, h, :],
                                             eglob[:, 512 * c:512 * c + 512],
                                             start=False, stop=True,
                                             skip_group_check=True)
                            zs = sb.tile([1, 512], F32, tag="zs", bufs=2,
                                         name=f"zs{Hh}_{h}_{c}")
                            nc.scalar.copy(zs[:], U[64:65, :])
                            zr = sb.tile([1, 512], F32, tag="zr", bufs=2,
                                         name=f"zr{Hh}_{h}_{c}")
                            with nc.allow_low_precision(reason="softmax recip"):
                                nc.vector.reciprocal_approx_fast(
                                    zr[:], zs[:])
                            pz = pss.tile([128, 512], F32, tag="pss", bufs=3,
                                          name=f"pz{Hh}_{h}_{c}")
                            nc.tensor.matmul(pz[0:64, :], onesr[:], zr[:],
                                             start=True, stop=True)
                            bc = sb.tile([64, 512], F32, tag="bc", bufs=2,
                                         name=f"bc{Hh}_{h}_{c}")
                            nc.scalar.copy(bc[:], pz[0:64, :])
                            nc.vector.tensor_mul(
                                attn[cc][64 * hh:64 * hh + 64,
                                         512 * c:512 * c + 512],
                                U[0:64, :], bc[:])

                        # p2 finalize for this half's global queries
                        m0 = 16 * Hh
                        rz2 = sb.tile([32, 1], F32, tag="rz2", bufs=2,
                                      name=f"rz2{Hh}_{h}")
                        with nc.allow_low_precision(reason="softmax recip"):
                            nc.vector.reciprocal(rz2[:], p2sb[h][:, 64:65])
                        p2n = sb.tile([32, 64], F32, tag="p2n", bufs=2,
                                      name=f"p2n{Hh}_{h}")
                        nc.vector.tensor_scalar_mul(p2n[:], p2sb[h][:, 0:64],
                                                    rz2[:])
                        pt = pss.tile([128, 512], F32, tag="pss", bufs=3,
                                      name=f"pt{Hh}_{h}")
                        nc.tensor.transpose(pt[0:64, 0:32], p2n[:], ident[:])
                        nc.vector.tensor_copy(
                            attn[cc][64 * hh:64 * hh + 64, 0::64],
                            pt[0:64, m0:m0 + 16])

                vaprev = va[7]

                # out projection for this half
                for tc_ in range(8):
                    for nck in range(2):
                        po = psu.tile([128, 512], F32, tag="psu", bufs=2,
                                      name=f"po{Hh}_{tc_}_{nck}")
                        for cci in range(4):
                            nc.tensor.matmul(
                                po[:], attn[cci][:, 128 * tc_:128 * tc_ + 128],
                                wo[:, cci, 512 * nck:512 * nck + 512],
                                start=(cci == 0), stop=(cci == 3))
                        os_ = sb.tile([128, 512], F32, tag="os", bufs=3,
                                      name=f"os{Hh}_{tc_}_{nck}")
                        nc.scalar.copy(os_[:], po[:])
                        nc.sync.dma_start(
                            out=out_d[t0 + 128 * tc_:t0 + 128 * tc_ + 128,
                                      512 * nck:512 * nck + 512],
                            in_=os_[:])

    nc.compile()
    return nc


def _prep_inputs(x, W_qkv, W_out):
    bf = ml_dtypes.bfloat16
    pos = np.arange(T, dtype=np.float32)[:, None]
    half = DH // 2
    inv_freq = 1.0 / (10000.0 ** (np.arange(half, dtype=np.float32) / half))
    ang = pos * inv_freq[None, :]
    cosT = np.cos(ang).T.astype(np.float32)   # [32, T]
    sinT = np.sin(ang).T.astype(np.float32)
    cos4 = np.ascontiguousarray(np.tile(cosT, (4, 1)))
    sin4 = np.ascontiguousarray(np.tile(sinT, (4, 1)))
    cosg = np.ascontiguousarray(cos4[:, ::64])
    sing = np.ascontiguousarray(sin4[:, ::64])

    s = np.arange(128)[:, None]
    u = np.arange(256)[None, :]
    mwin1 = ((u >= s) & ((u <= s + 127) | (s % 64 == 0))).astype(bf)
    mwin = np.concatenate([mwin1, mwin1], axis=1)
    mwin4 = np.ascontiguousarray(np.tile(mwin1, (1, 8)))
    m = np.arange(32)[:, None]
    q = np.arange(T)[None, :]
    mglob = (q >= 128 * (m // 2 + 2)).astype(bf)
    mglob4 = np.ascontiguousarray(np.tile(mglob, (4, 1)))
    sj = np.arange(128)[:, None, None]
    jj = np.arange(16)[None, :, None]
    mm_ = np.arange(32)[None, None, :]
    mp2 = (64 * mm_ >= 128 * jj + sj).astype(bf).reshape(128, 512)

    in_maps = []
    for core in range(NCORES):
        b, g = core // 2, core % 2
        xb = x[b]                                  # [T, D]
        xt = np.ascontiguousarray(
            xb.T.reshape(8, 128, T).transpose(1, 0, 2)).astype(bf)
        xg = np.ascontiguousarray(
            xb[0::64].T.reshape(8, 128, 32).transpose(1, 0, 2)).astype(bf)

        # wq/wk: [p, G, eo, dc, 32j+f]
        def pack_qk(Wsec):
            # Wsec: [1024, 512] = q or k columns for this core's 8 heads
            a = Wsec.reshape(8, 128, 2, 4, 32, 2)  # [dc, p, G, j, f, eo]
            a = a.transpose(1, 2, 5, 0, 3, 4)      # [p, G, eo, dc, j, f]
            return np.ascontiguousarray(a.reshape(128, 2, 2, 8, 128)).astype(bf)

        wq = pack_qk(W_qkv[:, 512 * g:512 * g + 512])
        wk = pack_qk(W_qkv[:, D + 512 * g:D + 512 * g + 512])
        wv = np.ascontiguousarray(
            W_qkv[:, 2 * D + 512 * g:2 * D + 512 * (g + 1)]
            .reshape(8, 128, 512).transpose(1, 0, 2)).astype(bf)
        wo = np.ascontiguousarray(
            W_out[512 * g:512 * (g + 1)]
            .reshape(4, 128, 1024).transpose(1, 0, 2)).astype(bf)
        in_maps.append({
            "xt": xt, "xg": xg, "wq": wq, "wk": wk, "wv": wv, "wo": wo,
            "cos4": cos4, "sin4": sin4, "cosg": cosg, "sing": sing,
            "mwin": mwin, "mwin4": mwin4, "mglob4": mglob4, "mp2": mp2,
        })
    return in_maps


def kernel(x, W_qkv, W_out, b_out):
    x = np.asarray(x, dtype=np.float32)
    W_qkv = np.asarray(W_qkv, dtype=np.float32)
    W_out = np.asarray(W_out, dtype=np.float32)
    b_out = np.asarray(b_out, dtype=np.float32)

    if "nc" not in _cache:
        _cache["nc"] = _build()
    nc = _cache["nc"]

    in_maps = _prep_inputs(x, W_qkv, W_out)
    res = run_bass_kernel_spmd(nc, in_maps, core_ids=list(range(NCORES)))

    out = np.zeros((B, T, D), dtype=np.float32)
    for core in range(NCORES):
        out[core // 2] += res.results[core]["out"]
    out += b_out[None, None, :]
    return out



# revision 6
# speedup vs baseline: 1.0660x; 1.0660x over previous
"""Trainium2 Bass kernel for nn_MultiHeadSelfAttention_36472862277880, v3.

Sparse attention (local window 128, global stride 64, causal) with RoPE.
Sharding: 8 cores = 4 batches x 2 head-groups (8 heads each core).

v3 vs v2:
- scores contract K=64 (E and O halves stacked in partitions) via an
  SBUF->SBUF DMA repack of q/k after rope -> half the PE accumulation
  rounds for window/global-key/global-query scores.
- softmax normalizer broadcast moved off the PE (was a ones-matmul) and
  off the scalar engine (psum copies) onto gpsimd partition_broadcast;
  reciprocal reads the U psum row directly.
- startup bubble fix: wv + xh chunk DMAs first, globals phase emitted
  after the V projection, wo/masks late.
- output partials stored bf16 (host accumulates in f32).

Self-contained: hardcodes all shapes; only imports the system concourse repo.
"""
import sys

if "/opt/trn_rl_repo" not in sys.path:
    sys.path.insert(0, "/opt/trn_rl_repo")

import numpy as np
import ml_dtypes

import concourse.bass as bass
import concourse.bacc as bacc
import concourse.tile as tile
from concourse import mybir
from concourse.bass_utils import run_bass_kernel_spmd
from concourse.masks import make_identity

F32 = mybir.dt.float32
F32R = mybir.dt.float32r
BF16 = mybir.dt.bfloat16

B, T, D, H, DH = 4, 2048, 1024, 16, 64
HALF = T // 2
NCORES = 8
EXP_SCALE = 0.125

_cache = {}


def _build():
    nc = bacc.Bacc("TRN2", target_bir_lowering=False, debug=False, num_devices=1)

    xt_d = nc.dram_tensor("xt", [128, 8, T], BF16, kind="ExternalInput").ap()
    xg_d = nc.dram_tensor("xg", [128, 8, 32], BF16, kind="ExternalInput").ap()
    wq_d = nc.dram_tensor("wq", [128, 2, 2, 8, 128], BF16, kind="ExternalInput").ap()
    wk_d = nc.dram_tensor("wk", [128, 2, 2, 8, 128], BF16, kind="ExternalInput").ap()
    wv_d = nc.dram_tensor("wv", [128, 8, 512], BF16, kind="ExternalInput").ap()
    wo_d = nc.dram_tensor("wo", [128, 4, 1024], BF16, kind="ExternalInput").ap()
    cos_d = nc.dram_tensor("cos4", [128, T], F32, kind="ExternalInput").ap()
    sin_d = nc.dram_tensor("sin4", [128, T], F32, kind="ExternalInput").ap()
    cosg_d = nc.dram_tensor("cosg", [128, 32], F32, kind="ExternalInput").ap()
    sing_d = nc.dram_tensor("sing", [128, 32], F32, kind="ExternalInput").ap()
    mwin_d = nc.dram_tensor("mwin", [128, 512], BF16, kind="ExternalInput").ap()
    mwin4_d = nc.dram_tensor("mwin4", [128, 2048], BF16, kind="ExternalInput").ap()
    mglob_d = nc.dram_tensor("mglob4", [128, T], BF16, kind="ExternalInput").ap()
    mp2_d = nc.dram_tensor("mp2", [128, 512], BF16, kind="ExternalInput").ap()
    out_d = nc.dram_tensor("out", [T, D], BF16, kind="ExternalOutput").ap()

    EXPF = mybir.ActivationFunctionType.Exp

    with tile.TileContext(nc) as tc:
        from contextlib import ExitStack
        with ExitStack() as ctx:
            sb = ctx.enter_context(tc.tile_pool(name="sb", bufs=1))
            psqk = ctx.enter_context(tc.tile_pool(name="psqk", bufs=1, space="PSUM"))
            pss = ctx.enter_context(tc.tile_pool(name="pss", bufs=1, space="PSUM"))
            psu = ctx.enter_context(tc.tile_pool(name="psu", bufs=1, space="PSUM"))

            # ---------- weights / inputs: priority-ordered DMAs ----------
            wv = sb.tile([128, 8, 512], BF16, name="wv")
            nc.sync.dma_start(out=wv[:], in_=wv_d)
            # first half of x, in two 512-token chunks so V proj starts early
            xh0 = [sb.tile([128, 8, 512], BF16, tag=f"xh{c}", bufs=2,
                           name=f"xh0_{c}") for c in range(2)]
            for c in range(2):
                nc.sync.dma_start(out=xh0[c][:],
                                  in_=xt_d[:, :, 512 * c:512 * c + 512])
            wq = sb.tile([128, 2, 2, 8, 128], BF16, name="wq")
            nc.sync.dma_start(out=wq[:], in_=wq_d)
            wk = sb.tile([128, 2, 2, 8, 128], BF16, name="wk")
            nc.sync.dma_start(out=wk[:], in_=wk_d)
            cos4 = sb.tile([128, T], F32, name="cos4")
            nc.sync.dma_start(out=cos4[:], in_=cos_d)
            sin4 = sb.tile([128, T], F32, name="sin4")
            nc.sync.dma_start(out=sin4[:], in_=sin_d)
            xg = sb.tile([128, 8, 32], BF16, name="xg")
            nc.sync.dma_start(out=xg[:], in_=xg_d)
            cosg = sb.tile([128, 32], F32, name="cosg")
            nc.sync.dma_start(out=cosg[:], in_=cosg_d)
            sing = sb.tile([128, 32], F32, name="sing")
            nc.sync.dma_start(out=sing[:], in_=sing_d)
            mwin = sb.tile([128, 512], BF16, name="mwin")
            nc.sync.dma_start(out=mwin[:], in_=mwin_d)
            mwin4 = sb.tile([128, 2048], BF16, name="mwin4")
            nc.sync.dma_start(out=mwin4[:], in_=mwin4_d)
            mglob4 = sb.tile([128, T], BF16, name="mglob4")
            nc.sync.dma_start(out=mglob4[:], in_=mglob_d)
            mp2 = sb.tile([128, 512], BF16, name="mp2")
            nc.sync.dma_start(out=mp2[:], in_=mp2_d)
            wo = sb.tile([128, 4, 1024], BF16, name="wo")
            nc.sync.dma_start(out=wo[:], in_=wo_d)

            ident = sb.tile([32, 32], F32, name="ident")
            make_identity(nc, ident[:])
            onesr = sb.tile([1, 64], F32, name="onesf")
            nc.vector.memset(onesr[:], 1.0)

            p2sb = [sb.tile([32, 65], F32, tag="p2sb", bufs=8, name=f"p2sb{h}")
                    for h in range(8)]
            ktbEO = {}
            vaprev = None
            globals_done = [False]
            vg = None
            qgEO, kgEO = {}, {}

            def emit_globals():
                # V at global tokens
                nonlocal vg
                pvg = pss.tile([128, 512], F32, tag="pss", bufs=3, name="pvg")
                for dc in range(8):
                    nc.tensor.matmul(pvg[0:32, :], xg[:, dc, :], wv[:, dc, :],
                                     start=(dc == 0), stop=(dc == 7))
                vg = sb.tile([32, 8, 65], BF16, name="vg")
                nc.scalar.copy(vg[:, :, 0:64],
                               pvg[0:32, :].rearrange("p (a b) -> p a b", a=8))
                nc.vector.memset(vg[:, :, 64], 1.0)

                # Q,K at global tokens: packed [128 = 4h x 32f, 32 gtok]
                pall = pss.tile([128, 16, 32], F32, tag="pss", bufs=3,
                                name="pall")
                for qk, wd in ((0, wq), (1, wk)):
                    for G in range(2):
                        for eo in range(2):
                            idx = 4 * qk + 2 * G + eo
                            for dc in range(8):
                                nc.tensor.matmul(
                                    pall[:, idx, :], wd[:, G, eo, dc, :],
                                    xg[:, dc, :],
                                    start=(idx == 0 and dc == 0),
                                    stop=(idx == 7 and dc == 7),
                                    skip_group_check=True)

                # rope + EO-combined repack ([128 = 2h x (32E|32O)], 32)
                rtg = [sb.tile([128, 32], F32, tag="rtg", bufs=4,
                               name=f"rtg{i}") for i in range(4)]
                for qk, Dst in ((0, qgEO), (1, kgEO)):
                    for G in range(2):
                        pE = pall[:, 4 * qk + 2 * G + 0, :]
                        pO = pall[:, 4 * qk + 2 * G + 1, :]
                        Et = sb.tile([128, 32], F32, tag="qgE", bufs=8,
                                     name=f"g{qk}{G}E")
                        Ot = sb.tile([128, 32], F32, tag="qgO", bufs=8,
                                     name=f"g{qk}{G}O")
                        t0_, t1_ = rtg[0], rtg[1]
                        nc.vector.tensor_mul(t0_[:], pE, cosg[:])
                        nc.vector.tensor_mul(t1_[:], pO, sing[:])
                        nc.vector.tensor_sub(Et[:], t0_[:], t1_[:])
                        t2_, t3_ = rtg[2], rtg[3]
                        nc.vector.tensor_mul(t2_[:], pE, sing[:])
                        nc.vector.tensor_mul(t3_[:], pO, cosg[:])
                        nc.vector.tensor_add(Ot[:], t2_[:], t3_[:])
                        for j in range(2):
                            P = 2 * G + j
                            t_ = sb.tile([128, 32], BF16, tag="gEO", bufs=8,
                                         name=f"gEO{qk}{P}")
                            for jj in range(2):
                                h2 = 2 * j + jj
                                nc.vector.tensor_copy(
                                    t_[64 * jj:64 * jj + 32, :],
                                    Et[32 * h2:32 * h2 + 32, :])
                                nc.vector.tensor_copy(
                                    t_[64 * jj + 32:64 * jj + 64, :],
                                    Ot[32 * h2:32 * h2 + 32, :])
                            Dst[P] = t_

            # ---------- half loop ----------
            xh_next = None
            for Hh in range(2):
                t0 = HALF * Hh

                xh = xh0 if Hh == 0 else xh_next

                # V projection
                va = []
                for jl in range(8):
                    xc, xoff = xh[jl // 4], 128 * (jl % 4)
                    pv = psu.tile([128, 512], F32, tag="psu", bufs=2,
                                  name=f"pv{Hh}_{jl}")
                    for dc in range(8):
                        nc.tensor.matmul(pv[:],
                                         xc[:, dc, xoff:xoff + 128],
                                         wv[:, dc, :],
                                         start=(dc == 0), stop=(dc == 7))
                    v = sb.tile([128, 8, 65], BF16, tag="va", bufs=9,
                                name=f"va{Hh}_{jl}")
                    nc.scalar.copy(v[:, :, 0:64],
                                   pv[:].rearrange("p (a b) -> p a b", a=8))
                    nc.vector.memset(v[:, :, 64], 1.0)
                    va.append(v)

                if not globals_done[0]:
                    emit_globals()
                    globals_done[0] = True

                # QK projection + rope -> qE/qO/kE/kO [128=4hx32f, 1024] bf16
                # then SBUF->SBUF DMA repack into qEO/kEO [128=2hx64, 1024]
                qE, qO, kE, kO = {}, {}, {}, {}
                qEO, kEO = {}, {}
                for G in range(2):
                    qE[G] = sb.tile([128, HALF], BF16, tag="qE", bufs=1,
                                    name=f"qE{Hh}_{G}")
                    qO[G] = sb.tile([128, HALF], BF16, tag="qO", bufs=1,
                                    name=f"qO{Hh}_{G}")
                    kE[G] = sb.tile([128, HALF], BF16, tag="kE", bufs=1,
                                    name=f"kE{Hh}_{G}")
                    kO[G] = sb.tile([128, HALF], BF16, tag="kO", bufs=1,
                                    name=f"kO{Hh}_{G}")
                    for tc_ in range(2):
                        sl = slice(512 * tc_, 512 * tc_ + 512)
                        cosS = cos4[:, t0 + 512 * tc_:t0 + 512 * tc_ + 512]
                        sinS = sin4[:, t0 + 512 * tc_:t0 + 512 * tc_ + 512]
                        for wd, Et, Ot in ((wq, qE, qO), (wk, kE, kO)):
                            pE = psqk.tile([128, 512], F32, tag="psqk", bufs=3,
                                           name=f"pE{Hh}_{G}_{tc_}_{id(wd)}")
                            pO = psqk.tile([128, 512], F32, tag="psqk", bufs=3,
                                           name=f"pO{Hh}_{G}_{tc_}_{id(wd)}")
                            for dc in range(8):
                                nc.tensor.matmul(pE[:], wd[:, G, 0, dc, :],
                                                 xh[tc_][:, dc, :],
                                                 start=(dc == 0), stop=(dc == 7))
                            for dc in range(8):
                                nc.tensor.matmul(pO[:], wd[:, G, 1, dc, :],
                                                 xh[tc_][:, dc, :],
                                                 start=(dc == 0), stop=(dc == 7))
                            ta = sb.tile([128, 512], F32, tag="rt", bufs=4,
                                         name=f"rta{Hh}_{G}_{tc_}")
                            tb = sb.tile([128, 512], F32, tag="rt", bufs=4,
                                         name=f"rtb{Hh}_{G}_{tc_}")
                            nc.vector.tensor_mul(ta[:], pE[:], cosS)
                            nc.vector.tensor_mul(tb[:], pO[:], sinS)
                            nc.vector.tensor_sub(Et[G][:, sl], ta[:], tb[:])
                            tc2 = sb.tile([128, 512], F32, tag="rt", bufs=4,
                                          name=f"rtc{Hh}_{G}_{tc_}")
                            td = sb.tile([128, 512], F32, tag="rt", bufs=4,
                                         name=f"rtd{Hh}_{G}_{tc_}")
                            nc.vector.tensor_mul(tc2[:], pE[:], sinS)
                            nc.vector.tensor_mul(td[:], pO[:], cosS)
                            nc.vector.tensor_add(Ot[G][:, sl], tc2[:], td[:])
                    # repack this G's 2 head-pairs into EO-combined tiles
                    for j in range(2):
                        P = 2 * G + j
                        qEO[P] = sb.tile([128, HALF], BF16, tag="qEO", bufs=2,
                                         name=f"qEO{Hh}_{P}")
                        kEO[P] = sb.tile([128, HALF], BF16, tag="kEO", bufs=2,
                                         name=f"kEO{Hh}_{P}")
                        for jj in range(2):
                            h2 = 2 * j + jj  # strip index within G tiles
                            ss = slice(32 * h2, 32 * h2 + 32)
                            r0 = 64 * jj
                            nc.sync.dma_start(out=qEO[P][r0:r0 + 32, :],
                                              in_=qE[G][ss, :])
                            nc.sync.dma_start(out=qEO[P][r0 + 32:r0 + 64, :],
                                              in_=qO[G][ss, :])
                            nc.sync.dma_start(out=kEO[P][r0:r0 + 32, :],
                                              in_=kE[G][ss, :])
                            nc.sync.dma_start(out=kEO[P][r0 + 32:r0 + 64, :],
                                              in_=kO[G][ss, :])

                if Hh == 0:
                    for P in range(4):
                        ktbEO[P] = sb.tile([128, 128], BF16, tag="ktbEO",
                                           bufs=4, name=f"ktbEO{P}")
                        nc.vector.tensor_copy(ktbEO[P][:],
                                              kEO[P][:, 896:1024])
                    # prefetch next half's x while scores run
                    xh_next = [sb.tile([128, 8, 512], BF16, tag=f"xh{c}",
                                       bufs=2, name=f"xh1_{c}")
                               for c in range(2)]
                    for c in range(2):
                        nc.sync.dma_start(
                            out=xh_next[c][:],
                            in_=xt_d[:, :, HALF + 512 * c:HALF + 512 * c + 512])

                attn = [sb.tile([128, HALF], BF16, tag="attn", bufs=8,
                                name=f"attn{Hh}_{cc}") for cc in range(4)]

                for h in range(8):
                    P = h // 2
                    cc, hh = h // 2, h % 2
                    r0 = 64 * (h % 2)
                    rs = slice(r0, r0 + 64)
                    tp = (r0, 0)

                    # window scores (merged exp buffer, single mask mul)
                    et4 = sb.tile([128, 2048], BF16, tag="et4", bufs=2,
                                  name=f"et4{Hh}_{h}")
                    for jp in range(4):
                        j0, j1 = 2 * jp, 2 * jp + 1
                        w1 = 256 if j1 < 7 else 128
                        pw = pss.tile([128, 512], F32, tag="pss", bufs=3,
                                      name=f"pw{Hh}_{h}_{jp}")
                        nc.tensor.matmul(
                            pw[:, 0:256],
                            kEO[P][rs, 128 * j0:128 * j0 + 128],
                            qEO[P][rs, 128 * j0:128 * j0 + 256],
                            start=True, stop=False, tile_position=tp,
                            skip_group_check=True)
                        nc.tensor.matmul(
                            pw[:, 256:256 + w1],
                            kEO[P][rs, 128 * j1:128 * j1 + 128],
                            qEO[P][rs, 128 * j1:128 * j1 + w1],
                            start=False, stop=True, tile_position=tp,
                            skip_group_check=True)
                        nc.scalar.activation(
                            et4[:, 512 * jp:512 * jp + 256 + w1],
                            pw[:, 0:256 + w1], EXPF, scale=EXP_SCALE)
                    ep4 = sb.tile([128, 2048], BF16, tag="ep4", bufs=2,
                                  name=f"ep4{Hh}_{h}")
                    nc.vector.tensor_mul(ep4[:, 0:1920], et4[:, 0:1920],
                                         mwin4[:, 0:1920])
                    ej = [ep4[:, 256 * i:256 * i + 256] for i in range(8)]

                    eb = None
                    if Hh == 1:
                        pb = pss.tile([128, 512], F32, tag="pss", bufs=3,
                                      name=f"pb{h}")
                        nc.tensor.matmul(pb[:, 0:128], ktbEO[P][rs, :],
                                         qEO[P][rs, 0:128],
                                         start=True, stop=True,
                                         tile_position=tp,
                                         skip_group_check=True)
                        etb = sb.tile([128, 512], BF16, tag="et", bufs=3,
                                      name=f"etb{h}")
                        nc.scalar.activation(etb[:, 0:128], pb[:, 0:128],
                                             EXPF, scale=EXP_SCALE)
                        eb = sb.tile([128, 128], BF16, tag="eb", bufs=2,
                                     name=f"eb{h}")
                        nc.vector.tensor_mul(eb[:], etb[:, 0:128],
                                             mwin[:, 128:256])

                    # global-key scores
                    eglob = sb.tile([32, HALF], BF16, tag="eglob", bufs=2,
                                    name=f"eg{Hh}_{h}")
                    etg = sb.tile([32, HALF], BF16, tag="etg", bufs=2,
                                  name=f"etg{Hh}_{h}")
                    for tc_ in range(2):
                        sl = slice(512 * tc_, 512 * tc_ + 512)
                        pg = pss.tile([128, 512], F32, tag="pss", bufs=3,
                                      name=f"pg{Hh}_{h}_{tc_}")
                        nc.tensor.matmul(pg[0:32, :], kgEO[P][rs, :],
                                         qEO[P][rs, sl],
                                         start=True, stop=True,
                                         tile_position=tp,
                                         skip_group_check=True)
                        nc.scalar.activation(etg[:, sl], pg[0:32, :],
                                             EXPF, scale=EXP_SCALE)
                    nc.vector.tensor_mul(
                        eglob[:], etg[:],
                        mglob4[0:32, t0:t0 + HALF])

                    # global-query (p2) scores
                    pp = pss.tile([128, 512], F32, tag="pss", bufs=3,
                                  name=f"pp{Hh}_{h}")
                    for jl in range(8):
                        nc.tensor.matmul(
                            pp[:, 32 * jl:32 * jl + 32],
                            kEO[P][rs, 128 * jl:128 * jl + 128],
                            qgEO[P][rs, :],
                            start=(jl == 0), stop=(jl == 7),
                            tile_position=tp, skip_group_check=True)
                    etp = sb.tile([128, 512], BF16, tag="et", bufs=3,
                                  name=f"etp{Hh}_{h}")
                    nc.scalar.activation(etp[:, 0:256], pp[:, 0:256],
                                         EXPF, scale=EXP_SCALE)
                    ep2 = sb.tile([128, 256], BF16, tag="ep2", bufs=2,
                                  name=f"ep2{Hh}_{h}")
                    nc.vector.tensor_mul(ep2[:], etp[:, 0:256],
                                         mp2[:, 256 * Hh:256 * Hh + 256])

                    pc = pss.tile([32, 65], F32, tag="pss", bufs=3,
                                  name=f"pc{Hh}_{h}")
                    for jl in range(8):
                        nc.tensor.matmul(pc[:], ep2[:, 32 * jl:32 * jl + 32],
                                         va[jl][:, h, :],
                                         start=(jl == 0), stop=(jl == 7))
                    if Hh == 0:
                        nc.vector.tensor_copy(p2sb[h][:], pc[:])
                    else:
                        nc.vector.tensor_add(p2sb[h][:], p2sb[h][:], pc[:])

                    # AV u-chunks
                    for c in range(2):
                        U = psu.tile([128, 512], F32, tag="psu", bufs=2,
                                     name=f"U{Hh}_{h}_{c}")
                        if c == 0:
                            parts = [(0, 0, 256, 0), (1, 128, 384, 0),
                                     (2, 256, 512, 0), (3, 384, 512, 0)]
                        else:
                            parts = [(4, 0, 256, 0), (3, 0, 128, 128),
                                     (5, 128, 384, 0), (6, 256, 512, 0),
                                     (7, 384, 512, 0)]
                        first = True
                        for (jl, u0, u1, e0c) in parts:
                            wdt = u1 - u0
                            nc.tensor.matmul(U[0:65, u0:u1],
                                             va[jl][:, h, :],
                                             ej[jl][:, e0c:e0c + wdt],
                                             start=first, stop=False,
                                             skip_group_check=True)
                            first = False
                        if Hh == 1 and c == 0:
                            nc.tensor.matmul(U[0:65, 0:128],
                                             vaprev[:, h, :], eb[:],
                                             start=False, stop=False,
                                             skip_group_check=True)
                        nc.tensor.matmul(U[0:65, :], vg[:, h, :],
                                         eglob[:, 512 * c:512 * c + 512],
                                         start=False, stop=True,
                                         skip_group_check=True)
                        zs = sb.tile([1, 512], F32, tag="zs", bufs=2,
                                     name=f"zs{Hh}_{h}_{c}")
                        nc.scalar.copy(zs[:], U[64:65, :])
                        zr = sb.tile([1, 512], F32, tag="zr", bufs=2,
                                     name=f"zr{Hh}_{h}_{c}")
                        with nc.allow_low_precision(reason="softmax recip"):
                            nc.vector.reciprocal_approx_fast(
                                zr[:], zs[:])
                        pz = pss.tile([128, 512], F32, tag="pss", bufs=3,
                                      name=f"pz{Hh}_{h}_{c}")
                        nc.tensor.matmul(pz[0:64, :], onesr[:], zr[:],
                                         start=True, stop=True)
                        bc = sb.tile([64, 512], F32, tag="bc", bufs=2,
                                     name=f"bc{Hh}_{h}_{c}")
                        nc.scalar.copy(bc[:], pz[0:64, :])
                        nc.vector.tensor_mul(
                            attn[cc][64 * hh:64 * hh + 64,
                                     512 * c:512 * c + 512],
                            U[0:64, :], bc[:])

                    # p2 finalize for this half's global queries
                    m0 = 16 * Hh
                    rz2 = sb.tile([32, 1], F32, tag="rz2", bufs=2,
                                  name=f"rz2{Hh}_{h}")
                    with nc.allow_low_precision(reason="softmax recip"):
                        nc.vector.reciprocal(rz2[:], p2sb[h][:, 64:65])
                    p2n = sb.tile([32, 64], F32, tag="p2n", bufs=2,
                                  name=f"p2n{Hh}_{h}")
                    nc.vector.tensor_scalar_mul(p2n[:], p2sb[h][:, 0:64],
                                                rz2[:])
                    pt = pss.tile([128, 512], F32, tag="pss", bufs=3,
                                  name=f"pt{Hh}_{h}")
                    nc.tensor.transpose(pt[0:64, 0:32], p2n[:], ident[:])
                    nc.vector.tensor_copy(
                        attn[cc][64 * hh:64 * hh + 64, 0::64],
                        pt[0:64, m0:m0 + 16])

                vaprev = va[7]

                # out projection for this half
                for tc_ in range(8):
                    for nck in range(2):
                        po = psu.tile([128, 512], F32, tag="psu", bufs=2,
                                      name=f"po{Hh}_{tc_}_{nck}")
                        for cci in range(4):
                            nc.tensor.matmul(
                                po[:], attn[cci][:, 128 * tc_:128 * tc_ + 128],
                                wo[:, cci, 512 * nck:512 * nck + 512],
                                start=(cci == 0), stop=(cci == 3))
                        os_ = sb.tile([128, 512], BF16, tag="os", bufs=3,
                                      name=f"os{Hh}_{tc_}_{nck}")
                        nc.scalar.copy(os_[:], po[:])
                        nc.sync.dma_start(
                            out=out_d[t0 + 128 * tc_:t0 + 128 * tc_ + 128,
                                      512 * nck:512 * nck + 512],
                            in_=os_[:])

    nc.compile()
    return nc


def _prep_inputs(x, W_qkv, W_out):
    bf = ml_dtypes.bfloat16
    pos = np.arange(T, dtype=np.float32)[:, None]
    half = DH // 2
    inv_freq = 1.0 / (10000.0 ** (np.arange(half, dtype=np.float32) / half))
    ang = pos * inv_freq[None, :]
    cosT = np.cos(ang).T.astype(np.float32)   # [32, T]
    sinT = np.sin(ang).T.astype(np.float32)
    cos4 = np.ascontiguousarray(np.tile(cosT, (4, 1)))
    sin4 = np.ascontiguousarray(np.tile(sinT, (4, 1)))
    cosg = np.ascontiguousarray(cos4[:, ::64])
    sing = np.ascontiguousarray(sin4[:, ::64])

    s = np.arange(128)[:, None]
    u = np.arange(256)[None, :]
    mwin1 = ((u >= s) & ((u <= s + 127) | (s % 64 == 0))).astype(bf)
    mwin = np.concatenate([mwin1, mwin1], axis=1)
    mwin4 = np.ascontiguousarray(np.tile(mwin1, (1, 8)))
    m = np.arange(32)[:, None]
    q = np.arange(T)[None, :]
    mglob = (q >= 128 * (m // 2 + 2)).astype(bf)
    mglob4 = np.ascontiguousarray(np.tile(mglob, (4, 1)))
    sj = np.arange(128)[:, None, None]
    jj = np.arange(16)[None, :, None]
    mm_ = np.arange(32)[None, None, :]
    mp2 = (64 * mm_ >= 128 * jj + sj).astype(bf).reshape(128, 512)

    in_maps = []
    for core in range(NCORES):
        b, g = core // 2, core % 2
        xb = x[b]                                  # [T, D]
        xt = np.ascontiguousarray(
            xb.T.reshape(8, 128, T).transpose(1, 0, 2)).astype(bf)
        xg = np.ascontiguousarray(
            xb[0::64].T.reshape(8, 128, 32).transpose(1, 0, 2)).astype(bf)

        # wq/wk: [p, G, eo, dc, 32j+f]
        def pack_qk(Wsec):
            # Wsec: [1024, 512] = q or k columns for this core's 8 heads
            a = Wsec.reshape(8, 128, 2, 4, 32, 2)  # [dc, p, G, j, f, eo]
            a = a.transpose(1, 2, 5, 0, 3, 4)      # [p, G, eo, dc, j, f]
            return np.ascontiguousarray(a.reshape(128, 2, 2, 8, 128)).astype(bf)

        wq = pack_qk(W_qkv[:, 512 * g:512 * g + 512])
        wk = pack_qk(W_qkv[:, D + 512 * g:D + 512 * g + 512])
        wv = np.ascontiguousarray(
            W_qkv[:, 2 * D + 512 * g:2 * D + 512 * (g + 1)]
            .reshape(8, 128, 512).transpose(1, 0, 2)).astype(bf)
        wo = np.ascontiguousarray(
            W_out[512 * g:512 * (g + 1)]
            .reshape(4, 128, 1024).transpose(1, 0, 2)).astype(bf)
        in_maps.append({
            "xt": xt, "xg": xg, "wq": wq, "wk": wk, "wv": wv, "wo": wo,
            "cos4": cos4, "sin4": sin4, "cosg": cosg, "sing": sing,
            "mwin": mwin, "mwin4": mwin4, "mglob4": mglob4, "mp2": mp2,
        })
    return in_maps


def kernel(x, W_qkv, W_out, b_out):
    x = np.asarray(x, dtype=np.float32)
    W_qkv = np.asarray(W_qkv, dtype=np.float32)
    W_out = np.asarray(W_out, dtype=np.float32)
    b_out = np.asarray(b_out, dtype=np.float32)

    if "nc" not in _cache:
        _cache["nc"] = _build()
    nc = _cache["nc"]

    in_maps = _prep_inputs(x, W_qkv, W_out)
    res = run_bass_kernel_spmd(nc, in_maps, core_ids=list(range(NCORES)))

    out = np.zeros((B, T, D), dtype=np.float32)
    for core in range(NCORES):
        out[core // 2] += res.results[core]["out"].astype(np.float32)
    out += b_out[None, None, :]
    return out
